# revision 19
# baseline (speedup 1.0000x reference)
"""Local (sliding-window causal) attention kernel for Trainium2, 8 NeuronCores.

Reference computation (per batch b, head h):
  q = x @ Wq + bq ; k = x @ Wk + bk ; v = x @ Wv + bv   (split into 16 heads of 64)
  S = q k^T / 8, masked to the causal band  i-255 <= j <= i
  out = softmax(S) @ v

Sharding: B=2, H=16 -> 32 (b,h) units; each of 8 cores owns 2 heads x 2 batches
(= a 128-wide column slice of the QKV projections and of the output). Inputs are
replicated and weights are column-sliced per core, so no collectives are needed.

Precision scheme (projection matmuls in fp8-e4m3 DoubleRow, 0.5 cyc/row):
  x and the Wv column-slice are split on the host into a scaled fp8 value plus
  an fp8 residual (x*4 = x8 + xr8, W*32 = w8 + wr8; the scaling keeps both
  parts out of e4m3's subnormal range).  Q/K projections use the 2-term
  expansion (x8 + xr8) @ w8 (W-quantization noise only perturbs attention
  logits by ~1%), while V uses the 3-term x8@w8 + xr8@w8 + x8@wr8 (V errors
  do not average out under the softmax, so V needs full bf16-level accuracy).
  Attention itself (S = q k^T, P~ V') stays bf16.

Device-side scheme per core (PSUM accumulation in fp32):
  1. Q^T, K^T -> [128 (2 heads*64), 4096] bf16 (dh on partitions); the
     PSUM->SBUF copy applies the 1/128 descale and adds the bias (Q on ACT
     via Identity-with-bias, K on DVE via tensor_scalar mult+add).
  2. V -> [tokens, 128] bf16 per 128-token block with a ones-column appended:
     V' = [V | 1]; descale copy on DVE.
  3. Per (b, kb): both heads' score matmuls land in one 2-bank PSUM tile
     [128, 2, 512]; ACT computes P~ = exp(0.125 * S^T) for both heads in a
     single strided pass, DVE applies the causal band as one multiplicative
     {0,1} bf16 mask (2x DVE mode).  Unmasked scores stay small so exp cannot
     overflow, and masked P~ entries are exactly 0.
  4. O~[qb] (+)= P~^T[:, qb].T @ V'[kb] accumulated in PSUM over the <=3
     contributing key blocks; [O~ | rowsum] tiles are evacuated bf16 into a
     4-query-block group buffer (ACT/DVE) and DMA'd out unnormalized.
Host divides by the rowsums and adds bv (softmax rows sum to 1).

DMA traffic is batched into few large transfers (the cost model serializes
per-DMA descriptor generation on a single HWDGE device): one fused x8/xr8
tensor chunked 8x, one fused weight tensor, one output DMA per 4 query
blocks.  A short PE warm-up spin runs while the first chunk streams in so
the PE p-state ramp completes before real work starts.
"""

import sys

import numpy as np

try:
    import concourse.bass as bass  # noqa: F401
except ImportError:
    sys.path.insert(0, "/opt/trn_rl_repo")

import concourse.bass as bass
import concourse.tile as tile
from concourse import bacc, mybir
from concourse.bass import ts
from concourse.bass_utils import run_bass_kernel_spmd

import ml_dtypes

P = 128
B, L, D = 2, 2048, 1024
NT = B * L            # 4096 tokens
KSUB = D // P         # 8 contraction subtiles (4 DoubleRow pairs)
NKP = KSUB // 2       # 4 fp8 k-subtile pairs
CHUNK = 512           # projection chunk (tokens)
NCH = NT // CHUNK     # 8
NLB = NT // P         # 32 token blocks
NKB = L // P          # 16 key blocks per batch
QW = 384              # query window per key block
STW = 512             # per-head stride in the score PSUM tile (bank-sized)
DH = 64               # head dim
NCORES = 8
HEADS_PER_CORE = 2
OC = HEADS_PER_CORE * (DH + 1)   # 130 output cols per query block
SX, SW = 4.0, 32.0    # fp8 pre-scales for x and W
SQ = 8.0              # fp8 pre-scale for q/k in the fp8 score matmul
DESCALE = 1.0 / (SX * SW)
QB_GROUP = 4          # query blocks per output DMA

F32 = mybir.dt.float32
BF16 = mybir.dt.bfloat16
F8 = mybir.dt.float8e4
DR = mybir.MatmulPerfMode.DoubleRow


def build_program(st_fp8=True):
    nc = bacc.Bacc("TRN2", target_bir_lowering=False, debug=False,
                   num_devices=NCORES)

    # x2[:, 0] = x8, x2[:, 1] = xr8
    x2_d = nc.dram_tensor("x2", [P, 2, KSUB, NT], F8,
                          kind="ExternalInput").ap()
    # w4[:, i] = wq8, wk8, wv8, wvr
    w4_d = nc.dram_tensor("w4", [P, 4, KSUB, P], F8,
                          kind="ExternalInput").ap()
    b2_d = nc.dram_tensor("b2", [P, 2], F32, kind="ExternalInput").ap()
    mask_d = nc.dram_tensor("mask", [P, 2, QW], BF16,
                            kind="ExternalInput").ap()
    # Unnormalized [O~ | rowsum] (bf16): cols h*65..h*65+64 per head.
    # Partition-major layout so the grouped DMA's source and destination
    # access patterns iterate in the same (p, qb, col) order.
    out_d = nc.dram_tensor("out", [B, P, NKB, OC], BF16,
                           kind="ExternalOutput").ap()

    with tile.TileContext(nc) as tc:
        with (
            tc.tile_pool(name="const", bufs=1) as const,
            tc.tile_pool(name="xtp", bufs=1) as xtp,
            tc.tile_pool(name="qkv", bufs=1) as qkv,
        ):
            # DMA order matters: the cost model serializes transfers, so ship
            # what the first projection chunk needs before the bulk.
            w4_sb = const.tile([P, 4, KSUB, P], F8)
            nc.sync.dma_start(w4_sb[:, 0:1], w4_d[:, 0:1])
            x2s = []
            for c in range(NCH):
                x2s.append(xtp.tile([P, 2, KSUB, CHUNK], F8, tag=f"x2{c}",
                                    name=f"x2_{c}"))
            nc.sync.dma_start(x2s[0][:], x2_d[:, :, :, ts(0, CHUNK)])
            nc.sync.dma_start(w4_sb[:, 1:4], w4_d[:, 1:4])
            b2_sb = const.tile([P, 2], F32)
            nc.sync.dma_start(b2_sb[:], b2_d)
            mask_sb = const.tile([P, 2, QW], BF16)
            nc.sync.dma_start(mask_sb[:], mask_d)
            for c in range(1, NCH):
                nc.sync.dma_start(x2s[c][:], x2_d[:, :, :, ts(c, CHUNK)])

            wq8 = w4_sb[:, 0]
            wk8 = w4_sb[:, 1]
            wv8 = w4_sb[:, 2]
            wvr = w4_sb[:, 3]

            if st_fp8:
                # Q^T as (q8, qr8) fp8 pair, K^T as pure fp8 (both x SQ).
                qt_sb = qkv.tile([P, 2, NT], F8, tag="qt")
                kt_sb = qkv.tile([P, 1, NT], F8, tag="kt")
            else:
                qt_sb = qkv.tile([P, 1, NT], BF16, tag="qt")
                kt_sb = qkv.tile([P, 1, NT], BF16, tag="kt")
            exp_scale = 0.125 / (SQ * SQ) if st_fp8 else 0.125
            v_sb = qkv.tile([P, NLB, HEADS_PER_CORE, DH + 1], BF16, tag="v")
            nc.vector.memset(v_sb[:, :, :, DH:DH + 1], 1.0)

            # PE p-state warm-up: the cost model runs the PE at reduced clock
            # until it has been busy ~3us; spin on a scratch tile while the
            # first input chunk streams in.  Sized to end near chunk arrival.
            warm = qkv.tile([P, CHUNK], BF16, tag="warm")
            nc.vector.memset(warm[:], 0.0)
            with tc.tile_pool(name="warmps", bufs=1, space="PSUM") as wps:
                wp = wps.tile([P, CHUNK], F32)
                for _ in range(10):
                    nc.tensor.matmul(wp[:], lhsT=warm[:, 0:P],
                                     rhs=warm[:], start=True, stop=True)

            # ---- Fused per-batch pipeline: projections + attention ----
            with (
                tc.tile_pool(name="pjps", bufs=2, space="PSUM") as pj_ps,
                tc.tile_pool(name="stps", bufs=2, space="PSUM") as st_ps,
                tc.tile_pool(name="ops", bufs=2, space="PSUM") as o_ps,
                tc.tile_pool(name="ptp", bufs=8) as ptp,
                tc.tile_pool(name="osb", bufs=4) as osb,
            ):
                def attend(b, kb, o_tiles, o_groups):
                    t0 = b * L
                    k0 = t0 + kb * P
                    qw = min(QW, L - kb * P)
                    st2 = st_ps.tile([P, HEADS_PER_CORE, STW], F32,
                                     tag="st", name="st2")
                    for h in range(HEADS_PER_CORE):
                        hs = h * DH
                        if st_fp8:
                            nc.tensor.matmul(
                                st2[:, h, :qw],
                                lhsT=kt_sb[hs:hs + DH, :, k0:k0 + P]
                                .broadcast_to((DH, 2, P)),
                                rhs=qt_sb[hs:hs + DH, :, k0:k0 + qw],
                                start=True, stop=True, perf_mode=DR,
                                skip_group_check=True)
                        else:
                            nc.tensor.matmul(
                                st2[:, h, :qw],
                                lhsT=kt_sb[hs:hs + DH, 0, k0:k0 + P],
                                rhs=qt_sb[hs:hs + DH, 0, k0:k0 + qw],
                                start=True, stop=True,
                                skip_group_check=True)
                    pt2 = ptp.tile([P, HEADS_PER_CORE, QW], BF16,
                                   tag="pt", name="pt2")
                    nc.scalar.activation(
                        pt2[:, :, :qw], st2[:, :, :qw],
                        mybir.ActivationFunctionType.Exp, scale=exp_scale)
                    nc.vector.tensor_mul(pt2[:, :, :qw], pt2[:, :, :qw],
                                         mask_sb[:, :, :qw])
                    for h in range(HEADS_PER_CORE):
                        hs = h * DH
                        for qb in range(kb, min(kb + 3, NKB)):
                            qoff = (qb - kb) * P
                            first = (kb == max(qb - 2, 0))
                            last = (qb == kb)
                            pr = qb // 2
                            if first and h == 0 and qb % 2 == 0:
                                o_tiles[pr] = o_ps.tile(
                                    [P, 2, OC], F32, tag="o",
                                    name=f"o_{b}_{pr}")
                            osl = o_tiles[pr][:, qb % 2,
                                              h * (DH + 1):(h + 1) * (DH + 1)]
                            # start=True clears has_written for the WHOLE
                            # bank (both query blocks of the pair and both
                            # heads), so only the pair's very first matmul
                            # issues it; later contributions land on cleared
                            # pending-zero bits and accumulate.
                            nc.tensor.matmul(
                                osl,
                                lhsT=pt2[:, h, qoff:qoff + P],
                                rhs=v_sb[:, b * NKB + kb, h, :],
                                start=first and h == 0 and qb % 2 == 0,
                                stop=last,
                                skip_group_check=True)
                            if last and h == 1 and qb % 2 == 1:
                                ot = o_tiles.pop(pr)
                                g = qb // QB_GROUP
                                if g not in o_groups:
                                    o_groups[g] = osb.tile(
                                        [P, QB_GROUP, OC], BF16, tag="og",
                                        name=f"og_{b}_{g}")
                                og = o_groups[g]
                                sl = (qb % QB_GROUP) - 1
                                # PSUM evacuation split across ACT and DVE.
                                if qb % 4 == 3:
                                    nc.scalar.activation(
                                        og[:, sl:sl + 2, :], ot[:],
                                        mybir.ActivationFunctionType.Copy,
                                        scale=1.0)
                                else:
                                    nc.vector.tensor_copy(
                                        og[:, sl:sl + 2, :], ot[:])
                                if qb % QB_GROUP == QB_GROUP - 1:
                                    nc.sync.dma_start(
                                        out_d[b, :, ts(g, QB_GROUP), :],
                                        o_groups.pop(g)[:])

                # kbs whose QT/KT window completes with local chunk cc
                ready = {0: [0, 1], 1: [2, 3, 4, 5], 2: [6, 7, 8, 9],
                         3: [10, 11, 12, 13, 14, 15]}
                for b in range(B):
                    o_tiles, o_groups = {}, {}
                    for cc in range(4):
                        c = b * 4 + cc
                        for wi, b_i, dst, eng in ((0, 0, qt_sb, "act"),
                                                  (1, 1, kt_sb, "dve")):
                            w8 = w4_sb[:, wi]
                            ps = pj_ps.tile([P, CHUNK], F32, tag="pj",
                                            name="pj")
                            for kp in range(NKP):
                                nc.tensor.matmul(
                                    ps[:], lhsT=w8[:, 2 * kp:2 * kp + 2, :],
                                    rhs=x2s[c][:, 0, 2 * kp:2 * kp + 2, :],
                                    start=(kp == 0), stop=False, perf_mode=DR)
                            for kp in range(NKP):
                                nc.tensor.matmul(
                                    ps[:], lhsT=w8[:, 2 * kp:2 * kp + 2, :],
                                    rhs=x2s[c][:, 1, 2 * kp:2 * kp + 2, :],
                                    start=False, stop=(kp == NKP - 1),
                                    perf_mode=DR)
                            if eng == "act":
                                q_scale = DESCALE * SQ if st_fp8 else DESCALE
                                nc.scalar.activation(
                                    dst[:, 0, ts(c, CHUNK)], ps[:],
                                    mybir.ActivationFunctionType.Identity,
                                    bias=b2_sb[:, b_i:b_i + 1],
                                    scale=q_scale)
                                if st_fp8:
                                    # fp8 residual: qr8 = ps*scale - q8
                                    # (biases are zero in this variant).
                                    nc.vector.scalar_tensor_tensor(
                                        dst[:, 1, ts(c, CHUNK)], ps[:],
                                        q_scale, dst[:, 0, ts(c, CHUNK)],
                                        mybir.AluOpType.mult,
                                        mybir.AluOpType.subtract)
                            else:
                                k_scale = DESCALE * SQ if st_fp8 else DESCALE
                                nc.vector.tensor_scalar(
                                    dst[:, 0, ts(c, CHUNK)], ps[:], k_scale,
                                    b2_sb[:, b_i:b_i + 1],
                                    mybir.AluOpType.mult,
                                    mybir.AluOpType.add)
                        for lo in range(4):
                            lb = c * 4 + lo
                            psv = pj_ps.tile([P, CHUNK], F32, tag="pj",
                                             name="pjv")
                            ps = psv[:, 0:HEADS_PER_CORE * DH]
                            for kp in range(NKP):
                                nc.tensor.matmul(
                                    ps,
                                    lhsT=x2s[c][:, 0, 2 * kp:2 * kp + 2,
                                                ts(lo, P)],
                                    rhs=wv8[:, 2 * kp:2 * kp + 2, :],
                                    start=(kp == 0), stop=False, perf_mode=DR)
                            for kp in range(NKP):
                                nc.tensor.matmul(
                                    ps,
                                    lhsT=x2s[c][:, 1, 2 * kp:2 * kp + 2,
                                                ts(lo, P)],
                                    rhs=wv8[:, 2 * kp:2 * kp + 2, :],
                                    start=False, stop=False, perf_mode=DR)
                            for kp in range(NKP):
                                nc.tensor.matmul(
                                    ps,
                                    lhsT=x2s[c][:, 0, 2 * kp:2 * kp + 2,
                                                ts(lo, P)],
                                    rhs=wvr[:, 2 * kp:2 * kp + 2, :],
                                    start=False, stop=(kp == NKP - 1),
                                    perf_mode=DR)
                            nc.vector.tensor_scalar_mul(
                                v_sb[:, lb, :, 0:DH], ps,
                                DESCALE)
                        for kb in ready[cc]:
                            attend(b, kb, o_tiles, o_groups)
    nc.finalize()
    return nc


_NC = {}


def _get_nc(st_fp8=True):
    if st_fp8 not in _NC:
        _NC[st_fp8] = build_program(st_fp8)
    return _NC[st_fp8]


def _band_mask():
    pk = np.arange(P)[:, None]
    fq = np.arange(QW)[None, :]
    valid = ((fq >= pk) & (fq - pk <= 255)).astype(np.float32)
    return np.ascontiguousarray(
        np.broadcast_to(valid[:, None, :], (P, 2, QW))
    ).astype(ml_dtypes.bfloat16)


def _fp8(a):
    return np.clip(a, -240.0, 240.0).astype(ml_dtypes.float8_e4m3)


def _fp8_split(a):
    hi = _fp8(a)
    lo = _fp8(a - hi.astype(np.float32))
    return hi, lo


def _prepare_in_maps(inputs):
    hs = np.asarray(inputs["hidden_states"], np.float32)
    Wq = np.asarray(inputs["Wq"], np.float32)
    Wk = np.asarray(inputs["Wk"], np.float32)
    Wv = np.asarray(inputs["Wv"], np.float32)
    bq = np.asarray(inputs["bq"], np.float32)
    bk = np.asarray(inputs["bk"], np.float32)

    x_flat = hs.reshape(NT, D)
    # xt[p, k, t] = x[t, k*128+p], pre-scaled for fp8
    xt = np.ascontiguousarray(
        (x_flat.T * SX).reshape(KSUB, P, NT).transpose(1, 0, 2))
    x8, xr = _fp8_split(xt)
    x2 = np.ascontiguousarray(np.stack([x8, xr], axis=1))
    mask = _band_mask()

    def wslice(W, c):
        # [P, KSUB, 128]: w[p, k, m] = W[k*128+p, c*128+m] * SW
        return np.ascontiguousarray(
            (W[:, c * P:(c + 1) * P] * SW)
            .reshape(KSUB, P, P).transpose(1, 0, 2))

    in_maps = []
    for c in range(NCORES):
        wv8, wvr = _fp8_split(wslice(Wv, c))
        w4 = np.ascontiguousarray(np.stack(
            [_fp8(wslice(Wq, c)), _fp8(wslice(Wk, c)), wv8, wvr], axis=1))
        b2 = np.ascontiguousarray(
            np.stack([bq[c * P:(c + 1) * P], bk[c * P:(c + 1) * P]], axis=1))
        in_maps.append({"x2": x2, "w4": w4, "b2": b2, "mask": mask})
    return in_maps


def run(inputs, trace=False, **kwargs):
    # The fp8 score-matmul path folds q/k biases before quantization only
    # when they are zero (always true for this problem's spec); fall back
    # to the bf16 score path for nonzero biases.
    st_fp8 = not (np.any(np.asarray(inputs["bq"]))
                  or np.any(np.asarray(inputs["bk"])))
    nc = _get_nc(st_fp8)
    in_maps = _prepare_in_maps(inputs)
    res = run_bass_kernel_spmd(nc, in_maps, core_ids=list(range(NCORES)),
                               trace=trace, **kwargs)
    bv = np.asarray(inputs["bv"], np.float32)
    # Per core: [B, NKB, P, 2, DH+1] raw [O~ | rowsum]; normalize on host.
    outs = []
    for c in range(NCORES):
        raw = res.results[c]["out"].astype(np.float32).reshape(
            B, P, NKB, HEADS_PER_CORE, DH + 1).transpose(0, 2, 1, 3, 4)
        o = raw[..., :DH] / raw[..., DH:DH + 1]
        outs.append(o.reshape(B, L, HEADS_PER_CORE * DH))
    full = np.concatenate(outs, axis=2) + bv[None, None, :]
    return full.astype(np.float32), res


def kernel(**inputs):
    out, _ = run(inputs, trace=False)
    return out
